# revision 32
# baseline (speedup 1.0000x reference)
"""VQ-VAE forward (nn_Autoencoder_VQVAE) on 8 Trainium2 NeuronCores.

Strategy: data-parallel over batch (128 rows/core). Activations live in SBUF
as (128 partitions = channel%128, C//128 chunks, N free) with N = pos*128+b.
All convs are PE GEMMs accumulating over (cin-chunk, kernel tap) in PSUM.
Training-mode BatchNorm needs full-batch stats: local bn_stats/bn_aggr ->
tiny 8-core AllReduce of (sum, sumsq) per channel -> fused BN+ReLU applied
straight from PSUM via one scalar-engine activation pass. VQ stats
(histogram + commitment-loss partial) ride the 4th AllReduce.

Encoder + VQ are fp32 (argmin margins require it); decoder optionally fp16.
"""
import numpy as np

T, B, D, H, KC = 30, 1024, 135, 512, 512
NCORES = 8
BL = B // NCORES            # 128 batch rows per core
NB = BL                     # free-dim block size
HC = H // 128               # 4 channel chunks
EPS = 1e-5

DEC_FP16 = True             # decoder matmuls in fp16 (4x PE throughput)

CHV = {n: i for i, n in enumerate(
    ["b_in", "g_e1", "b_e1", "g_e2", "b_e2", "g_e3", "b_e3", "b_q",
     "g_d1", "b_d1", "g_d2", "b_d2", "g_d3", "b_d3"])}

_CACHE = {}


def _statsplit(n):
    out = []
    while n > 0:
        out.append(min(512, n))
        n -= out[-1]
    return out


def _nsplits(tout):
    # split tout blocks of NB fp32 into <=512-elem (= 1 PSUM bank) regions
    full = 512 // NB
    out = []
    lo = 0
    while lo < tout:
        hi = min(lo + full, tout)
        out.append((lo, hi))
        lo = hi
    return out


def _build():
    import contextlib
    import concourse.bass as bass
    import concourse.tile as tile
    from concourse import bacc, mybir
    from concourse.masks import make_identity

    f32 = mybir.dt.float32
    dec_dt = mybir.dt.float16 if DEC_FP16 else f32
    nc = bacc.Bacc(None, target_bir_lowering=False, debug=False,
                   num_devices=NCORES)

    # ---- DRAM I/O ----
    xin = nc.dram_tensor("xin", [D, T * NB], f32, kind="ExternalInput")
    w_in = nc.dram_tensor("w_in", [D, H], f32, kind="ExternalInput")
    chvec = nc.dram_tensor("chvec", [len(CHV), HC, 128], f32, kind="ExternalInput")
    bout_d = nc.dram_tensor("b_out", [D, 1], f32, kind="ExternalInput")
    wenc = {}
    for name, kk in [("w_e1", 5), ("w_e2", 3), ("w_e3", 2)]:
        wenc[name] = nc.dram_tensor(name, [H, H, kk], f32, kind="ExternalInput")
    cb_d = nc.dram_tensor("codebook", [KC, H], f32, kind="ExternalInput")
    cbt_d = nc.dram_tensor("codebook_t", [H, KC], f32, kind="ExternalInput")
    wq_d = nc.dram_tensor("w_q", [H, H], dec_dt, kind="ExternalInput")
    wdec = {}
    for name, kk in [("w_d1", 2), ("w_d2", 3), ("w_d3", 5)]:
        wdec[name] = nc.dram_tensor(name, [H, H, kk], dec_dt, kind="ExternalInput")
    wout_d = nc.dram_tensor("w_out", [H, D], dec_dt, kind="ExternalInput")

    recon_d = nc.dram_tensor("recon", [D, T * NB], f32, kind="ExternalOutput")
    vq_d = nc.dram_tensor("vq", [1, 2], f32, kind="ExternalOutput")

    AX = mybir.AxisListType.X
    AF = mybir.ActivationFunctionType
    ALU = mybir.AluOpType

    with tile.TileContext(nc) as tc, contextlib.ExitStack() as ctx:
        consts = ctx.enter_context(tc.tile_pool(name="consts", bufs=1))
        wbig = ctx.enter_context(tc.tile_pool(name="wbig", bufs=1))
        wsm = ctx.enter_context(tc.tile_pool(name="wsm", bufs=1))
        acts = ctx.enter_context(tc.tile_pool(name="acts", bufs=1))
        rec = ctx.enter_context(tc.tile_pool(name="rec", bufs=3))
        small = ctx.enter_context(tc.tile_pool(name="small", bufs=2))
        dram = ctx.enter_context(tc.tile_pool(name="dram", bufs=2, space="DRAM"))
        psp = ctx.enter_context(tc.tile_pool(name="ps", bufs=4, space="PSUM"))

        _sc = [None]

        def scope(name):
            if _sc[0] is not None:
                nc.leave_named_scope(_sc[0][0], _sc[0][1], False)
            sid, _ = nc.enter_named_scope(name, False)
            _sc[0] = (name, sid)

        def wload(dram_t, kk, dt=f32):
            t = wbig.tile([128, HC, H, kk], dt, tag="w")
            nc.sync.dma_start(t[:, :, :, :],
                              dram_t.rearrange("(c p) o k -> p c o k", p=128))
            return t

        # ---------- constants ----------
        chv = consts.tile([128, len(CHV), HC], f32, tag="chv")
        nc.sync.dma_start(chv[:, :, :], chvec.rearrange("v c p -> p v c"))
        bout0 = consts.tile([128, 1], f32, tag="bout0")
        bout1 = consts.tile([7, 1], f32, tag="bout1")
        nc.sync.dma_start(bout0[:, :], bout_d[0:128, :])
        nc.sync.dma_start(bout1[:, :], bout_d[128:135, :])
        epst = consts.tile([128, 1], f32, tag="epst")
        nc.vector.memset(epst[:, :], EPS)
        ones1 = consts.tile([1, NB], f32, tag="ones1")
        nc.vector.memset(ones1[:, :], 1.0)
        onesP = consts.tile([128, 1], f32, tag="onesP")
        nc.vector.memset(onesP[:, :], 1.0)
        ioti = acts.tile([128, KC], mybir.dt.int32, tag="score")
        nc.gpsimd.iota(ioti[:, :], pattern=[[1, KC]], base=0, channel_multiplier=0)
        iotf = consts.tile([128, KC], f32, tag="iotf")
        nc.vector.tensor_copy(iotf[:, :], ioti[:, :])
        ident = consts.tile([128, 128], f32, tag="ident")
        make_identity(nc, ident[:, :])

        # PE warm-up: dep-free matmuls while input DMAs are in flight
        wu_ps = psp.tile([128, 128], f32, tag="big")
        for _ in range(48):
            nc.tensor.matmul(wu_ps[:, :], ident[:, :], ident[:, :],
                             start=True, stop=True)

        # collectives warm-up: absorb one-time CC/algorithm init during L0,
        # one per (kind, size) actually used later
        for wuw in (8, 13):
            wu_in = dram.tile([128, wuw], f32, tag=f"wuin{wuw}")
            wu_out = dram.tile([NCORES * 128, wuw], f32, tag=f"wuout{wuw}")
            nc.sync.dma_start(wu_in[:, :], ident[:, 0:wuw])
            nc.gpsimd.collective_compute(
                "AllGather", ALU.bypass, replica_groups=[list(range(NCORES))],
                ins=[wu_in[:, :].opt()], outs=[wu_out[:, :].opt()])

        def chvs(name):
            return chv[:, CHV[name], :]      # (128, HC)

        # ---------- small weights ----------
        win_t = wsm.tile([128, 2 * H], f32, tag="wina")
        nc.gpsimd.dma_start(win_t[:, 0:H], w_in[0:128, :])
        nc.gpsimd.dma_start(win_t[0:7, H:2 * H], w_in[128:135, :])

        # ---------- helpers ----------
        def _flat2d(ap):
            shp = ap.shape
            if len(shp) == 3:
                return ap.rearrange("p a b -> p (a b)")
            if len(shp) == 4:
                return ap.rearrange("p a b c -> p (a b c)")
            return ap

        def stats_to_pay(ps_tiles, pay, nfree):
            # pay[:, c] = sum over free of psum chunk c; pay[:, 4+c] = sumsq
            sqsc = acts.tile([128, nfree], f32, tag="sqsc")
            for c in range(HC):
                ap = _flat2d(ps_tiles[c][:])
                nc.vector.reduce_sum(pay[:, c:c + 1], ap, axis=AX)
                nc.scalar.activation(sqsc[:, :], ap,
                                     AF.Square, accum_out=pay[:, 4 + c:5 + c])

        def gather_pay(pay, W):
            din = dram.tile([128, W], f32, tag=f"arin{W}")
            dout = dram.tile([NCORES * 128, W], f32, tag=f"arout{W}")
            nc.gpsimd.dma_start(din[:, :], pay[:, :])
            nc.gpsimd.collective_compute(
                "AllGather", ALU.bypass,
                replica_groups=[list(range(NCORES))],
                ins=[din[:, :].opt()], outs=[dout[:, :].opt()])
            # preload the Sqrt LUT while the collective runs
            sqwarm = small.tile([128, 1], f32, tag="sqwarm")
            nc.scalar.activation(sqwarm[:, :], epst[:, :], AF.Sqrt,
                                 bias=epst[:, 0:1])
            gpay8 = small.tile([128, NCORES, W], f32, tag="gpay8")
            nc.sync.dma_start(gpay8[:, :, :],
                              dout.rearrange("(r p) w -> p r w", p=128))
            gpay = small.tile([128, W], f32, tag="gpay")
            nc.vector.reduce_sum(gpay[:, :],
                                 gpay8.rearrange("p r w -> p w r"), axis=AX)
            return gpay

        def bn_finalize(gpay, nglob, gname, bname):
            mean = small.tile([128, HC], f32, tag="mean")
            var = small.tile([128, HC], f32, tag="var")
            nc.scalar.mul(mean[:, :], gpay[:, 0:4], 1.0 / nglob)
            nc.scalar.mul(var[:, :], gpay[:, 4:8], 1.0 / nglob)
            msq = small.tile([128, HC], f32, tag="msq")
            nc.vector.tensor_mul(msq[:, :], mean[:, :], mean[:, :])
            nc.vector.tensor_sub(var[:, :], var[:, :], msq[:, :])
            nc.scalar.activation(var[:, :], var[:, :], AF.Sqrt, bias=epst[:, 0:1])
            nc.vector.reciprocal(var[:, :], var[:, :])
            scale = small.tile([128, HC], f32, tag="scale")
            bias = small.tile([128, HC], f32, tag="bias")
            nc.vector.tensor_mul(scale[:, :], var[:, :], chvs(gname))
            nc.vector.tensor_mul(bias[:, :], mean[:, :], scale[:, :])
            nc.vector.tensor_sub(bias[:, :], chvs(bname), bias[:, :])
            return scale, bias

        # ================= L0: h0 = W_in.T @ x + b_in =================
        scope("L0")
        h0 = acts.tile([128, HC, T * NB], f32, tag="big1")
        NL0 = 480
        HNB = T * NB // 2
        for half in range(2):
            xa = acts.tile([128, HNB], f32, tag="mid1")
            xb = acts.tile([7, HNB], f32, tag="mid2x")
            hsl = slice(half * HNB, (half + 1) * HNB)
            nc.gpsimd.dma_start(xa[:, :], xin[0:128, hsl])
            nc.gpsimd.dma_start(xb[:, :], xin[128:135, hsl])
            for m in range(HC):
                for n in range(HNB // NL0):
                    pt = psp.tile([128, NL0], f32, tag="big")
                    sl = slice(n * NL0, (n + 1) * NL0)
                    osl = slice(half * HNB + n * NL0, half * HNB + (n + 1) * NL0)
                    nc.tensor.matmul(pt[:, :], win_t[:, m * 128:(m + 1) * 128],
                                     xa[:, sl], start=True, stop=False)
                    nc.tensor.matmul(pt[:, :], win_t[0:7, H + m * 128:H + (m + 1) * 128],
                                     xb[:, sl], start=False, stop=True)
                    nc.vector.tensor_scalar_add(out=h0[:, m, osl], in0=pt[:, :],
                                                scalar1=chv[:, CHV["b_in"], m:m + 1])

        # ================= encoder convs =================
        def conv_enc(src, wtile, kk, tout, lname, out_tag):
            ps = []
            for m in range(HC):
                pt = psp.tile([128, tout * NB], f32, tag="big")
                for c in range(HC):
                    rsrc = src[:, c, :].rearrange(
                        "p (t k b) -> p t k b", t=tout, k=kk)
                    for k in range(kk):
                        for lo, hi in _nsplits(tout):
                            nc.tensor.matmul(
                                pt[:, lo * NB:hi * NB],
                                wtile[:, c, m * 128:(m + 1) * 128, k],
                                rsrc[:, lo:hi, k, :],
                                start=(c == 0) and (k == 0),
                                stop=(c == HC - 1) and (k == kk - 1))
                ps.append(pt)
            out = acts.tile([128, HC, tout * NB], f32, tag=out_tag)
            pay = small.tile([128, 8], f32, tag="pay")
            stats_to_pay(ps, pay, tout * NB)
            gpay = gather_pay(pay, 8)
            scale, bias = bn_finalize(gpay, float(NCORES * tout * NB),
                                      f"g_{lname}", f"b_{lname}")
            for m in range(HC):
                nc.scalar.activation(out[:, m, :], ps[m][:, :], AF.Relu,
                                     bias=bias[:, m:m + 1],
                                     scale=scale[:, m:m + 1])
            return out

        scope("e1")
        y1 = conv_enc(h0, wload(wenc["w_e1"], 5), 5, 6, "e1", "mid1")
        cb = wsm.tile([128, HC, H], f32, tag="cb")
        nc.sync.dma_start(cb[:, :, :], cb_d.rearrange("(c p) h -> p c h", p=128))
        cbt = wsm.tile([128, HC, KC], f32, tag="cbt")
        nc.sync.dma_start(cbt[:, :, :], cbt_d.rearrange("(c p) k -> p c k", p=128))
        wq = wsm.tile([128, HC, H], dec_dt, tag="wq")
        nc.sync.dma_start(wq[:, :, :], wq_d.rearrange("(c p) h -> p c h", p=128))
        wout = wsm.tile([128, HC, D], dec_dt, tag="wout")
        nc.sync.dma_start(wout[:, :, :], wout_d.rearrange("(c p) d -> p c d", p=128))

        # -|c_k|^2/2 as a (1, KC) row for the score bias matmul
        nrm = small.tile([128, HC], f32, tag="nrm")
        for c in range(HC):
            cbsq = acts.tile([128, H], f32, tag="score")
            nc.scalar.activation(cbsq[:, :], cb[:, c, :], AF.Square,
                                 accum_out=nrm[:, c:c + 1])
        nc.scalar.mul(nrm[:, :], nrm[:, :], -0.5)
        nrm_ps = psp.tile([HC, 128], f32, tag="big")
        nc.tensor.transpose(nrm_ps[:, :], nrm[:, :], ident[:, :])
        nrm_sb = small.tile([HC, 128], f32, tag="nrm_sb")
        nc.vector.tensor_copy(nrm_sb[:, :], nrm_ps[:, :])
        negnorm = consts.tile([1, KC], f32, tag="negnorm")
        for c in range(HC):
            nc.sync.dma_start(negnorm[0:1, c * 128:(c + 1) * 128],
                              nrm_sb[c:c + 1, :])

        scope("e2")
        y2 = conv_enc(y1, wload(wenc["w_e2"], 3), 3, 2, "e2", "a_e2")

        scope("e3")
        # e3 -> z (128, HC, NB)
        we3 = wload(wenc["w_e3"], 2)
        ps3 = []
        for m in range(HC):
            pt = psp.tile([128, NB], f32, tag="big")
            idx = 0
            for c in range(HC):
                for k in range(2):
                    nc.tensor.matmul(pt[:, :],
                                     we3[:, c, m * 128:(m + 1) * 128, k],
                                     y2[:, c, k * NB:(k + 1) * NB],
                                     start=(idx == 0), stop=(idx == 2 * HC - 1))
                    idx += 1
            ps3.append(pt)
        z = acts.tile([128, HC, NB], f32, tag="z")
        pay = small.tile([128, 8], f32, tag="pay")
        stats_to_pay(ps3, pay, NB)
        gpay = gather_pay(pay, 8)
        scale, bias = bn_finalize(gpay, float(NCORES * NB), "g_e3", "b_e3")
        for m in range(HC):
            nc.scalar.activation(z[:, m, :], ps3[m][:, :], AF.Relu,
                                 bias=bias[:, m:m + 1], scale=scale[:, m:m + 1])

        # ================= VQ =================
        scope("vq")
        sc_ps = psp.tile([128, KC], f32, tag="big")
        for c in range(HC):
            nc.tensor.matmul(sc_ps[:, :], z[:, c, :], cbt[:, c, :],
                             start=(c == 0), stop=False)
        nc.tensor.matmul(sc_ps[:, :], ones1[:, :], negnorm[:, :],
                         start=False, stop=True)
        score = acts.tile([128, KC], f32, tag="score")
        nc.vector.tensor_copy(score[:, :], sc_ps[:, :])
        mx8 = small.tile([128, 8], f32, tag="mx8")
        mi8 = small.tile([128, 8], mybir.dt.uint32, tag="mi8")
        nc.vector.max(mx8[:, :], score[:, :])
        nc.vector.max_index(mi8[:, :], mx8[:, :], score[:, :])
        idxf = small.tile([128, 1], f32, tag="idxf")
        nc.vector.tensor_copy(idxf[:, :], mi8[:, 0:1])
        oh = acts.tile([128, KC], f32, tag="oh")
        nc.vector.tensor_scalar(out=oh[:, :], in0=iotf[:, :],
                                scalar1=idxf[:, 0:1], scalar2=None,
                                op0=ALU.is_equal)
        oht = acts.tile([128, HC, NB], f32, tag="oht")
        hist = small.tile([128, HC], f32, tag="hist")
        for c in range(HC):
            tp = psp.tile([128, 128], f32, tag="big")
            nc.tensor.transpose(tp[:, :], oh[:, c * 128:(c + 1) * 128], ident[:, :])
            nc.vector.tensor_copy(oht[:, c, :], tp[:, :])
            nc.vector.reduce_sum(hist[:, c:c + 1], oht[:, c, :], axis=AX)
        qt = acts.tile([128, HC, NB], f32, tag="qt")
        if DEC_FP16:
            qt16 = acts.tile([128, HC, NB], dec_dt, tag="qt16")
        else:
            qt16 = qt
        for m in range(HC):
            qp = psp.tile([128, NB], f32, tag="big")
            for c in range(HC):
                nc.tensor.matmul(qp[:, :], cb[:, c, m * 128:(m + 1) * 128],
                                 oht[:, c, :], start=(c == 0), stop=(c == HC - 1))
            nc.vector.tensor_copy(qt[:, m, :], qp[:, :])
            if DEC_FP16:
                nc.vector.tensor_copy(qt16[:, m, :], qp[:, :])
        diff = acts.tile([128, HC, NB], f32, tag="diff")
        nc.vector.tensor_sub(diff[:, :, :], qt[:, :, :], z[:, :, :])
        vqcol = small.tile([128, 1], f32, tag="vqcol")
        nc.scalar.activation(diff[:, :, :], diff[:, :, :], AF.Square,
                             accum_out=vqcol[:, 0:1])

        # ================= decoder =================
        scope("dec12")
        d0 = acts.tile([128, HC, NB], dec_dt, tag="d0")
        for m in range(HC):
            dp = psp.tile([128, NB], f32, tag="big")
            for c in range(HC):
                nc.tensor.matmul(dp[:, :], wq[:, c, m * 128:(m + 1) * 128],
                                 qt16[:, c, :], start=(c == 0), stop=(c == HC - 1))
            nc.vector.tensor_scalar_add(out=d0[:, m, :], in0=dp[:, :],
                                        scalar1=chv[:, CHV["b_q"], m:m + 1])

        def conv_dec(src, wtile, kk, tin, lname, out_tag, extra=None, extra_w=0):
            # psum laid out (k, l, b) so one matmul covers all tin l-positions
            tout = tin * kk
            ps = []
            for m in range(HC):
                pt = psp.tile([128, kk, tin, NB], f32, tag="big")
                for k in range(kk):
                    for c in range(HC):
                        nc.tensor.matmul(
                            pt[:, k, :, :],
                            wtile[:, c, m * 128:(m + 1) * 128, k],
                            src[:, c, 0:tin * NB],
                            start=(c == 0), stop=(c == HC - 1))
                ps.append(pt)
            out = acts.tile([128, HC, tout * NB], dec_dt, tag=out_tag)
            W = 8 + extra_w
            pay = small.tile([128, W], f32, tag="pay")
            stats_to_pay(ps, pay, tout * NB)
            if extra is not None:
                nc.vector.tensor_copy(pay[:, 8:W], extra)
            gpay = gather_pay(pay, W)
            scale, bias = bn_finalize(gpay, float(NCORES * tout * NB),
                                      f"g_{lname}", f"b_{lname}")
            for m in range(HC):
                oap = out[:, m, :].rearrange("p (l k b) -> p k l b",
                                             l=tin, k=kk)
                nc.scalar.activation(oap, ps[m][:, :, :], AF.Relu,
                                     bias=bias[:, m:m + 1],
                                     scale=scale[:, m:m + 1])
            return out, gpay

        wd1 = wload(wdec["w_d1"], 2, dec_dt)
        extra = small.tile([128, 5], f32, tag="extra4")
        nc.vector.tensor_copy(extra[:, 0:4], hist[:, :])
        nc.vector.tensor_copy(extra[:, 4:5], vqcol[:, :])
        d1, gpay4 = conv_dec(d0, wd1, 2, 1, "d1", "mid2", extra=extra[:, :], extra_w=5)
        hist_g = small.tile([128, HC], f32, tag="hist_g")
        vq_g = small.tile([128, 1], f32, tag="vq_g")
        nc.vector.tensor_copy(hist_g[:, :], gpay4[:, 8:12])
        nc.vector.tensor_copy(vq_g[:, :], gpay4[:, 12:13])

        wd2 = wload(wdec["w_d2"], 3, dec_dt)
        d2, _ = conv_dec(d1, wd2, 3, 2, "d2", "mid1")

        # ---- d3: too big for PSUM residence; stats read PSUM, raw -> SBUF ----
        scope("d3")
        wd3 = wload(wdec["w_d3"], 5, dec_dt)
        d3raw = acts.tile([128, HC, T * NB], dec_dt, tag="big1")  # reuse h0 slot
        sump3 = consts.tile([128, HC, 3, 3], f32, tag="st12")
        sqp3 = consts.tile([128, HC, 3, 3], f32, tag="mv43")
        for m in range(HC):
            d3r_m = d3raw[:, m, :].rearrange("p (l k b) -> p k l b", l=6, k=5)
            for lp in range(3):          # l-pairs (2 l's per matmul, N=256)
                lsl = slice(2 * lp, 2 * lp + 2)
                rsrc = d2[:, :, 2 * lp * NB:(2 * lp + 2) * NB]
                pt_a = psp.tile([128, 2, 2, NB], f32, tag="big")
                pt_b = psp.tile([128, 2, 2, NB], f32, tag="big")
                pt_c = psp.tile([128, 1, 2, NB], f32, tag="big")
                pts = [pt_a, pt_b, pt_c]
                for k in range(5):
                    dst = pts[k // 2][:, k % 2, :, :]
                    for c in range(HC):
                        nc.tensor.matmul(
                            dst,
                            wd3[:, c, m * 128:(m + 1) * 128, k],
                            rsrc[:, c, :],
                            start=(c == 0), stop=(c == HC - 1))
                for j, pt in enumerate(pts):
                    nk = pt.shape[1]
                    sqsc3 = acts.tile([128, 2 * 2 * NB], f32, tag="sqsc")
                    nc.scalar.activation(sqsc3[:, 0:nk * 2 * NB], pt[:, :, :, :],
                                         AF.Square,
                                         accum_out=sqp3[:, m, lp, j:j + 1])
                    nc.vector.tensor_scalar(
                        out=d3r_m[:, 2 * j:2 * j + nk, lsl, :],
                        in0=pt[:, :, :, :],
                        scalar1=0.0, scalar2=0.0,
                        op0=ALU.add, op1=ALU.add,
                        accum_out=sump3[:, m, lp, j:j + 1])
        pay = small.tile([128, 8], f32, tag="pay")
        AXY = mybir.AxisListType.XY
        for m in range(HC):
            nc.vector.tensor_reduce(pay[:, m:m + 1], sump3[:, m, :, :],
                                    axis=AXY, op=ALU.add)
            nc.vector.tensor_reduce(pay[:, 4 + m:5 + m], sqp3[:, m, :, :],
                                    axis=AXY, op=ALU.add)
        gpay = gather_pay(pay, 8)
        scale, bias = bn_finalize(gpay, float(NCORES * T * NB), "g_d3", "b_d3")
        d3 = d3raw

        # ---- recon = W_out.T @ d3 + b_out (apply sliced n-major so the
        #      recon matmuls chase the BN applies) ----
        scope("recon")
        NR = 480
        for n in range(T * NB // NR):
            sl = slice(n * NR, (n + 1) * NR)
            for m in range(HC):
                nc.scalar.activation(d3[:, m, sl], d3raw[:, m, sl], AF.Relu,
                                     bias=bias[:, m:m + 1],
                                     scale=scale[:, m:m + 1])
            for mlo, msz, bt in [(0, 128, bout0), (128, 7, bout1)]:
                rp = psp.tile([128, NR], f32, tag="big")
                for c in range(HC):
                    nc.tensor.matmul(rp[0:msz, :], wout[:, c, mlo:mlo + msz],
                                     d3[:, c, sl],
                                     start=(c == 0), stop=(c == HC - 1))
                rs = rec.tile([128, NR], f32, tag="recsb")
                nc.vector.tensor_scalar_add(out=rs[0:msz, :], in0=rp[0:msz, :],
                                            scalar1=bt[:, 0:1])
                nc.sync.dma_start(recon_d[mlo:mlo + msz, sl], rs[0:msz, :])

        # ---- scalars: vq_loss & perplexity ----
        vps = psp.tile([1, 1], f32, tag="big")
        nc.tensor.matmul(vps[:, :], vq_g[:, 0:1], onesP[:, :], start=True, stop=True)
        scl = small.tile([1, 2], f32, tag="scl")
        nc.scalar.mul(scl[0:1, 0:1], vps[:, :], 1.25 / float(B * H))
        hist_gc = rec.tile([128, HC], f32, tag="recsb")
        nc.vector.tensor_copy(hist_gc[:, :], hist_g[:, :])
        p_t = small.tile([128, HC], f32, tag="p_t")
        lnp = small.tile([128, HC], f32, tag="lnp")
        nc.scalar.mul(p_t[:, :], hist_gc[:, :], 1.0 / float(B))
        eps10 = small.tile([128, 1], f32, tag="eps10")
        nc.vector.memset(eps10[:, :], 1e-10)
        nc.scalar.activation(lnp[:, :], hist_gc[:, :], AF.Ln,
                             bias=eps10[:, 0:1], scale=1.0 / float(B))
        nc.vector.tensor_mul(p_t[:, :], p_t[:, :], lnp[:, :])
        ent = small.tile([128, 1], f32, tag="ent")
        nc.vector.reduce_sum(ent[:, 0:1], p_t[:, :], axis=AX)
        entp = psp.tile([1, 1], f32, tag="big")
        nc.tensor.matmul(entp[:, :], ent[:, 0:1], onesP[:, :], start=True, stop=True)
        nc.scalar.activation(scl[0:1, 1:2], entp[:, :], AF.Exp, scale=-1.0)
        nc.sync.dma_start(vq_d[:, :], scl[0:1, :])
        scope("end")
        nc.leave_named_scope(_sc[0][0], _sc[0][1], False)

    nc.compile()
    return nc


def _prep_inputs(inputs):
    f = np.float32
    dec = np.float16 if DEC_FP16 else np.float32
    x = np.asarray(inputs["input_seqs"], f)
    chvec = np.stack([np.asarray(inputs[n], f).reshape(HC, 128)
                      for n in CHV]).astype(f)
    common = {
        "w_in": np.ascontiguousarray(np.asarray(inputs["W_in"], f)),
        "chvec": np.ascontiguousarray(chvec),
        "b_out": np.ascontiguousarray(np.asarray(inputs["b_out"], f).reshape(D, 1)),
        "codebook": np.ascontiguousarray(np.asarray(inputs["codebook"], f)),
        "codebook_t": np.ascontiguousarray(np.asarray(inputs["codebook"], f).T),
        "w_q": np.ascontiguousarray(np.asarray(inputs["W_q"], f).astype(dec)),
        "w_out": np.ascontiguousarray(np.asarray(inputs["W_out"], f).astype(dec)),
        "w_e1": np.ascontiguousarray(np.asarray(inputs["w_e1"], f).transpose(1, 0, 2)),
        "w_e2": np.ascontiguousarray(np.asarray(inputs["w_e2"], f).transpose(1, 0, 2)),
        "w_e3": np.ascontiguousarray(np.asarray(inputs["w_e3"], f).transpose(1, 0, 2)),
        "w_d1": np.ascontiguousarray(np.asarray(inputs["w_d1"], f).astype(dec)),
        "w_d2": np.ascontiguousarray(np.asarray(inputs["w_d2"], f).astype(dec)),
        "w_d3": np.ascontiguousarray(np.asarray(inputs["w_d3"], f).astype(dec)),
    }
    in_maps = []
    for c in range(NCORES):
        xc = np.ascontiguousarray(
            x[:, c * BL:(c + 1) * BL, :].transpose(2, 0, 1).reshape(D, T * BL))
        m = {"xin": xc}
        m.update(common)
        in_maps.append(m)
    return in_maps


def kernel(**inputs):
    from concourse.bass_utils import run_bass_kernel_spmd
    if "nc" not in _CACHE:
        _CACHE["nc"] = _build()
    nc = _CACHE["nc"]
    in_maps = _prep_inputs(inputs)
    for attempt in range(3):
        res = run_bass_kernel_spmd(nc, in_maps, core_ids=list(range(NCORES)))
        recon = np.empty((B, T, D), np.float32)
        for c in range(NCORES):
            rc = res.results[c]["recon"].reshape(D, T, BL)
            recon[c * BL:(c + 1) * BL] = rc.transpose(2, 1, 0)
        vqs = np.stack([res.results[c]["vq"] for c in range(NCORES)])
        # integrity: scalars are AllReduced, must agree on every core;
        # NaNs indicate a transient device fault -> rerun
        ok = (np.isfinite(recon).all() and np.isfinite(vqs).all()
              and all(np.array_equal(vqs[c], vqs[0]) for c in range(NCORES)))
        if ok:
            break
    vq = vqs[0]
    return recon, np.float32(vq[0, 0]), np.float32(vq[0, 1])


# revision 33
# speedup vs baseline: 1.0827x; 1.0827x over previous
"""VQ-VAE forward (nn_Autoencoder_VQVAE) on 8 Trainium2 NeuronCores.

Strategy: data-parallel over batch (128 rows/core). Activations live in SBUF
as (128 partitions = channel%128, C//128 chunks, N free) with N = pos*128+b.
All convs are PE GEMMs accumulating over (cin-chunk, kernel tap) in PSUM.
Training-mode BatchNorm needs full-batch stats: local bn_stats/bn_aggr ->
tiny 8-core AllReduce of (sum, sumsq) per channel -> fused BN+ReLU applied
straight from PSUM via one scalar-engine activation pass. VQ stats
(histogram + commitment-loss partial) ride the 4th AllReduce.

Encoder + VQ are fp32 (argmin margins require it); decoder optionally fp16.
"""
import numpy as np

T, B, D, H, KC = 30, 1024, 135, 512, 512
NCORES = 8
BL = B // NCORES            # 128 batch rows per core
NB = BL                     # free-dim block size
HC = H // 128               # 4 channel chunks
EPS = 1e-5

DEC_FP16 = True             # decoder matmuls in fp16 (4x PE throughput)

CHV = {n: i for i, n in enumerate(
    ["b_in", "g_e1", "b_e1", "g_e2", "b_e2", "g_e3", "b_e3", "b_q",
     "g_d1", "b_d1", "g_d2", "b_d2", "g_d3", "b_d3"])}

_CACHE = {}


def _statsplit(n):
    out = []
    while n > 0:
        out.append(min(512, n))
        n -= out[-1]
    return out


def _nsplits(tout):
    # split tout blocks of NB fp32 into <=512-elem (= 1 PSUM bank) regions
    full = 512 // NB
    out = []
    lo = 0
    while lo < tout:
        hi = min(lo + full, tout)
        out.append((lo, hi))
        lo = hi
    return out


def _build():
    import contextlib
    import concourse.bass as bass
    import concourse.tile as tile
    from concourse import bacc, mybir
    from concourse.masks import make_identity

    f32 = mybir.dt.float32
    dec_dt = mybir.dt.float16 if DEC_FP16 else f32
    nc = bacc.Bacc(None, target_bir_lowering=False, debug=False,
                   num_devices=NCORES)

    # ---- DRAM I/O ----
    xin = nc.dram_tensor("xin", [D, T * NB], f32, kind="ExternalInput")
    w_in = nc.dram_tensor("w_in", [D, H], f32, kind="ExternalInput")
    chvec = nc.dram_tensor("chvec", [len(CHV), HC, 128], f32, kind="ExternalInput")
    bout_d = nc.dram_tensor("b_out", [D, 1], f32, kind="ExternalInput")
    wenc = {}
    for name, kk in [("w_e1", 5), ("w_e2", 3), ("w_e3", 2)]:
        wenc[name] = nc.dram_tensor(name, [H, H, kk], f32, kind="ExternalInput")
    cb_d = nc.dram_tensor("codebook", [KC, H], f32, kind="ExternalInput")
    cbt_d = nc.dram_tensor("codebook_t", [H, KC], f32, kind="ExternalInput")
    wq_d = nc.dram_tensor("w_q", [H, H], dec_dt, kind="ExternalInput")
    wdec = {}
    for name, kk in [("w_d1", 2), ("w_d2", 3), ("w_d3", 5)]:
        wdec[name] = nc.dram_tensor(name, [H, H, kk], dec_dt, kind="ExternalInput")
    wout_d = nc.dram_tensor("w_out", [H, D], dec_dt, kind="ExternalInput")

    recon_d = nc.dram_tensor("recon", [D, T * NB], f32, kind="ExternalOutput")
    vq_d = nc.dram_tensor("vq", [1, 2], f32, kind="ExternalOutput")

    AX = mybir.AxisListType.X
    AF = mybir.ActivationFunctionType
    ALU = mybir.AluOpType

    with tile.TileContext(nc) as tc, contextlib.ExitStack() as ctx:
        consts = ctx.enter_context(tc.tile_pool(name="consts", bufs=1))
        wbig = ctx.enter_context(tc.tile_pool(name="wbig", bufs=1))
        wsm = ctx.enter_context(tc.tile_pool(name="wsm", bufs=1))
        acts = ctx.enter_context(tc.tile_pool(name="acts", bufs=1))
        rec = ctx.enter_context(tc.tile_pool(name="rec", bufs=3))
        small = ctx.enter_context(tc.tile_pool(name="small", bufs=2))
        dram = ctx.enter_context(tc.tile_pool(name="dram", bufs=2, space="DRAM"))
        psp = ctx.enter_context(tc.tile_pool(name="ps", bufs=4, space="PSUM"))

        _sc = [None]

        def scope(name):
            if _sc[0] is not None:
                nc.leave_named_scope(_sc[0][0], _sc[0][1], False)
            sid, _ = nc.enter_named_scope(name, False)
            _sc[0] = (name, sid)

        def wload(dram_t, kk, dt=f32):
            t = wbig.tile([128, HC, H, kk], dt, tag="w")
            nc.sync.dma_start(t[:, :, :, :],
                              dram_t.rearrange("(c p) o k -> p c o k", p=128))
            return t

        # ---------- constants ----------
        chv = consts.tile([128, len(CHV), HC], f32, tag="chv")
        nc.sync.dma_start(chv[:, :, :], chvec.rearrange("v c p -> p v c"))
        bout0 = consts.tile([128, 1], f32, tag="bout0")
        bout1 = consts.tile([7, 1], f32, tag="bout1")
        nc.sync.dma_start(bout0[:, :], bout_d[0:128, :])
        nc.sync.dma_start(bout1[:, :], bout_d[128:135, :])
        epst = consts.tile([128, 1], f32, tag="epst")
        nc.vector.memset(epst[:, :], EPS)
        ones1 = consts.tile([1, NB], f32, tag="ones1")
        nc.vector.memset(ones1[:, :], 1.0)
        onesP = consts.tile([128, 1], f32, tag="onesP")
        nc.vector.memset(onesP[:, :], 1.0)
        ioti = acts.tile([128, KC], mybir.dt.int32, tag="score")
        nc.gpsimd.iota(ioti[:, :], pattern=[[1, KC]], base=0, channel_multiplier=0)
        iotf = consts.tile([128, KC], f32, tag="iotf")
        nc.vector.tensor_copy(iotf[:, :], ioti[:, :])
        ident = consts.tile([128, 128], f32, tag="ident")
        make_identity(nc, ident[:, :])

        # PE warm-up: dep-free matmuls while input DMAs are in flight
        wu_ps = psp.tile([128, 128], f32, tag="big")
        for _ in range(48):
            nc.tensor.matmul(wu_ps[:, :], ident[:, :], ident[:, :],
                             start=True, stop=True)

        # collectives warm-up: absorb one-time CC/algorithm init during L0,
        # one per (kind, size) actually used later
        for wuw in (8, 13):
            wu_in = dram.tile([128, wuw], f32, tag=f"wuin{wuw}")
            wu_out = dram.tile([NCORES * 128, wuw], f32, tag=f"wuout{wuw}")
            nc.sync.dma_start(wu_in[:, :], ident[:, 0:wuw])
            nc.gpsimd.collective_compute(
                "AllGather", ALU.bypass, replica_groups=[list(range(NCORES))],
                ins=[wu_in[:, :].opt()], outs=[wu_out[:, :].opt()])

        def chvs(name):
            return chv[:, CHV[name], :]      # (128, HC)

        # ---------- small weights ----------
        win_t = wsm.tile([128, 2 * H], f32, tag="wina")
        nc.sync.dma_start(win_t[:, 0:H], w_in[0:128, :])
        nc.sync.dma_start(win_t[0:7, H:2 * H], w_in[128:135, :])

        # ---------- helpers ----------
        def _flat2d(ap):
            shp = ap.shape
            if len(shp) == 3:
                return ap.rearrange("p a b -> p (a b)")
            if len(shp) == 4:
                return ap.rearrange("p a b c -> p (a b c)")
            return ap

        def stats_to_pay(ps_tiles, pay, nfree):
            # pay[:, c] = sum over free of psum chunk c; pay[:, 4+c] = sumsq
            sqsc = acts.tile([128, nfree], f32, tag="sqsc")
            for c in range(HC):
                ap = _flat2d(ps_tiles[c][:])
                nc.vector.reduce_sum(pay[:, c:c + 1], ap, axis=AX)
                nc.scalar.activation(sqsc[:, :], ap,
                                     AF.Square, accum_out=pay[:, 4 + c:5 + c])

        def gather_pay(pay, W):
            din = dram.tile([128, W], f32, tag=f"arin{W}")
            dout = dram.tile([NCORES * 128, W], f32, tag=f"arout{W}")
            nc.gpsimd.dma_start(din[:, :], pay[:, :])
            nc.gpsimd.collective_compute(
                "AllGather", ALU.bypass,
                replica_groups=[list(range(NCORES))],
                ins=[din[:, :].opt()], outs=[dout[:, :].opt()])
            # preload the Sqrt LUT while the collective runs
            sqwarm = small.tile([128, 1], f32, tag="sqwarm")
            nc.scalar.activation(sqwarm[:, :], epst[:, :], AF.Sqrt,
                                 bias=epst[:, 0:1])
            gpay8 = small.tile([128, NCORES, W], f32, tag="gpay8")
            nc.sync.dma_start(gpay8[:, :, :],
                              dout.rearrange("(r p) w -> p r w", p=128))
            gpay = small.tile([128, W], f32, tag="gpay")
            nc.vector.reduce_sum(gpay[:, :],
                                 gpay8.rearrange("p r w -> p w r"), axis=AX)
            return gpay

        def bn_finalize(gpay, nglob, gname, bname):
            mean = small.tile([128, HC], f32, tag="mean")
            var = small.tile([128, HC], f32, tag="var")
            nc.scalar.mul(mean[:, :], gpay[:, 0:4], 1.0 / nglob)
            nc.scalar.mul(var[:, :], gpay[:, 4:8], 1.0 / nglob)
            msq = small.tile([128, HC], f32, tag="msq")
            nc.vector.tensor_mul(msq[:, :], mean[:, :], mean[:, :])
            nc.vector.tensor_sub(var[:, :], var[:, :], msq[:, :])
            nc.scalar.activation(var[:, :], var[:, :], AF.Sqrt, bias=epst[:, 0:1])
            nc.vector.reciprocal(var[:, :], var[:, :])
            scale = small.tile([128, HC], f32, tag="scale")
            bias = small.tile([128, HC], f32, tag="bias")
            nc.vector.tensor_mul(scale[:, :], var[:, :], chvs(gname))
            nc.vector.tensor_mul(bias[:, :], mean[:, :], scale[:, :])
            nc.vector.tensor_sub(bias[:, :], chvs(bname), bias[:, :])
            return scale, bias

        # ================= L0: h0 = W_in.T @ x + b_in =================
        scope("L0")
        h0 = acts.tile([128, HC, T * NB], f32, tag="big1")
        NL0 = 480
        HNB = T * NB // 2
        for half in range(2):
            xa = acts.tile([128, HNB], f32, tag="mid1")
            xb = acts.tile([7, HNB], f32, tag="mid2x")
            hsl = slice(half * HNB, (half + 1) * HNB)
            nc.sync.dma_start(xa[:, :], xin[0:128, hsl])
            nc.sync.dma_start(xb[:, :], xin[128:135, hsl])
            for m in range(HC):
                for n in range(HNB // NL0):
                    pt = psp.tile([128, NL0], f32, tag="big")
                    sl = slice(n * NL0, (n + 1) * NL0)
                    osl = slice(half * HNB + n * NL0, half * HNB + (n + 1) * NL0)
                    nc.tensor.matmul(pt[:, :], win_t[:, m * 128:(m + 1) * 128],
                                     xa[:, sl], start=True, stop=False)
                    nc.tensor.matmul(pt[:, :], win_t[0:7, H + m * 128:H + (m + 1) * 128],
                                     xb[:, sl], start=False, stop=True)
                    nc.vector.tensor_scalar_add(out=h0[:, m, osl], in0=pt[:, :],
                                                scalar1=chv[:, CHV["b_in"], m:m + 1])

        # ================= encoder convs =================
        def conv_enc(src, wtile, kk, tout, lname, out_tag):
            ps = []
            for m in range(HC):
                pt = psp.tile([128, tout * NB], f32, tag="big")
                for lo, hi in _nsplits(tout):
                    first = True
                    for c in range(HC):
                        rsrc = src[:, c, :].rearrange(
                            "p (t k b) -> p t k b", t=tout, k=kk)
                        for k in range(kk):
                            nc.tensor.matmul(
                                pt[:, lo * NB:hi * NB],
                                wtile[:, c, m * 128:(m + 1) * 128, k],
                                rsrc[:, lo:hi, k, :],
                                start=first,
                                stop=(c == HC - 1) and (k == kk - 1))
                            first = False
                ps.append(pt)
            out = acts.tile([128, HC, tout * NB], f32, tag=out_tag)
            pay = small.tile([128, 8], f32, tag="pay")
            stats_to_pay(ps, pay, tout * NB)
            gpay = gather_pay(pay, 8)
            scale, bias = bn_finalize(gpay, float(NCORES * tout * NB),
                                      f"g_{lname}", f"b_{lname}")
            for m in range(HC):
                nc.scalar.activation(out[:, m, :], ps[m][:, :], AF.Relu,
                                     bias=bias[:, m:m + 1],
                                     scale=scale[:, m:m + 1])
            return out

        scope("e1")
        y1 = conv_enc(h0, wload(wenc["w_e1"], 5), 5, 6, "e1", "mid1")
        cb = wsm.tile([128, HC, H], f32, tag="cb")
        nc.sync.dma_start(cb[:, :, :], cb_d.rearrange("(c p) h -> p c h", p=128))
        cbt = wsm.tile([128, HC, KC], f32, tag="cbt")
        nc.sync.dma_start(cbt[:, :, :], cbt_d.rearrange("(c p) k -> p c k", p=128))
        wq = wsm.tile([128, HC, H], dec_dt, tag="wq")
        nc.sync.dma_start(wq[:, :, :], wq_d.rearrange("(c p) h -> p c h", p=128))
        wout = wsm.tile([128, HC, D], dec_dt, tag="wout")
        nc.sync.dma_start(wout[:, :, :], wout_d.rearrange("(c p) d -> p c d", p=128))

        # -|c_k|^2/2 as a (1, KC) row for the score bias matmul
        nrm = small.tile([128, HC], f32, tag="nrm")
        for c in range(HC):
            cbsq = acts.tile([128, H], f32, tag="score")
            nc.scalar.activation(cbsq[:, :], cb[:, c, :], AF.Square,
                                 accum_out=nrm[:, c:c + 1])
        nc.scalar.mul(nrm[:, :], nrm[:, :], -0.5)
        nrm_ps = psp.tile([HC, 128], f32, tag="big")
        nc.tensor.transpose(nrm_ps[:, :], nrm[:, :], ident[:, :])
        nrm_sb = small.tile([HC, 128], f32, tag="nrm_sb")
        nc.vector.tensor_copy(nrm_sb[:, :], nrm_ps[:, :])
        negnorm = consts.tile([1, KC], f32, tag="negnorm")
        for c in range(HC):
            nc.sync.dma_start(negnorm[0:1, c * 128:(c + 1) * 128],
                              nrm_sb[c:c + 1, :])

        scope("e2")
        y2 = conv_enc(y1, wload(wenc["w_e2"], 3), 3, 2, "e2", "a_e2")

        scope("e3")
        # e3 -> z (128, HC, NB)
        we3 = wload(wenc["w_e3"], 2)
        ps3 = []
        for m in range(HC):
            pt = psp.tile([128, NB], f32, tag="big")
            idx = 0
            for c in range(HC):
                for k in range(2):
                    nc.tensor.matmul(pt[:, :],
                                     we3[:, c, m * 128:(m + 1) * 128, k],
                                     y2[:, c, k * NB:(k + 1) * NB],
                                     start=(idx == 0), stop=(idx == 2 * HC - 1))
                    idx += 1
            ps3.append(pt)
        z = acts.tile([128, HC, NB], f32, tag="z")
        pay = small.tile([128, 8], f32, tag="pay")
        stats_to_pay(ps3, pay, NB)
        gpay = gather_pay(pay, 8)
        scale, bias = bn_finalize(gpay, float(NCORES * NB), "g_e3", "b_e3")
        for m in range(HC):
            nc.scalar.activation(z[:, m, :], ps3[m][:, :], AF.Relu,
                                 bias=bias[:, m:m + 1], scale=scale[:, m:m + 1])

        # ================= VQ =================
        scope("vq")
        sc_ps = psp.tile([128, KC], f32, tag="big")
        for c in range(HC):
            nc.tensor.matmul(sc_ps[:, :], z[:, c, :], cbt[:, c, :],
                             start=(c == 0), stop=False)
        nc.tensor.matmul(sc_ps[:, :], ones1[:, :], negnorm[:, :],
                         start=False, stop=True)
        score = acts.tile([128, KC], f32, tag="score")
        nc.vector.tensor_copy(score[:, :], sc_ps[:, :])
        mx8 = small.tile([128, 8], f32, tag="mx8")
        mi8 = small.tile([128, 8], mybir.dt.uint32, tag="mi8")
        nc.vector.max(mx8[:, :], score[:, :])
        nc.vector.max_index(mi8[:, :], mx8[:, :], score[:, :])
        idxf = small.tile([128, 1], f32, tag="idxf")
        nc.vector.tensor_copy(idxf[:, :], mi8[:, 0:1])
        oh = acts.tile([128, KC], f32, tag="oh")
        nc.vector.tensor_scalar(out=oh[:, :], in0=iotf[:, :],
                                scalar1=idxf[:, 0:1], scalar2=None,
                                op0=ALU.is_equal)
        oht = acts.tile([128, HC, NB], f32, tag="oht")
        hist = small.tile([128, HC], f32, tag="hist")
        for c in range(HC):
            tp = psp.tile([128, 128], f32, tag="big")
            nc.tensor.transpose(tp[:, :], oh[:, c * 128:(c + 1) * 128], ident[:, :])
            nc.vector.tensor_copy(oht[:, c, :], tp[:, :])
            nc.vector.reduce_sum(hist[:, c:c + 1], oht[:, c, :], axis=AX)
        qt = acts.tile([128, HC, NB], f32, tag="qt")
        if DEC_FP16:
            qt16 = acts.tile([128, HC, NB], dec_dt, tag="qt16")
        else:
            qt16 = qt
        for m in range(HC):
            qp = psp.tile([128, NB], f32, tag="big")
            for c in range(HC):
                nc.tensor.matmul(qp[:, :], cb[:, c, m * 128:(m + 1) * 128],
                                 oht[:, c, :], start=(c == 0), stop=(c == HC - 1))
            nc.vector.tensor_copy(qt[:, m, :], qp[:, :])
            if DEC_FP16:
                nc.vector.tensor_copy(qt16[:, m, :], qp[:, :])
        diff = acts.tile([128, HC, NB], f32, tag="diff")
        nc.vector.tensor_sub(diff[:, :, :], qt[:, :, :], z[:, :, :])
        vqcol = small.tile([128, 1], f32, tag="vqcol")
        nc.scalar.activation(diff[:, :, :], diff[:, :, :], AF.Square,
                             accum_out=vqcol[:, 0:1])

        # ================= decoder =================
        scope("dec12")
        d0 = acts.tile([128, HC, NB], dec_dt, tag="d0")
        for m in range(HC):
            dp = psp.tile([128, NB], f32, tag="big")
            for c in range(HC):
                nc.tensor.matmul(dp[:, :], wq[:, c, m * 128:(m + 1) * 128],
                                 qt16[:, c, :], start=(c == 0), stop=(c == HC - 1))
            nc.vector.tensor_scalar_add(out=d0[:, m, :], in0=dp[:, :],
                                        scalar1=chv[:, CHV["b_q"], m:m + 1])

        def conv_dec(src, wtile, kk, tin, lname, out_tag, extra=None, extra_w=0):
            # psum laid out (k, l, b) so one matmul covers all tin l-positions
            tout = tin * kk
            ps = []
            for m in range(HC):
                pt = psp.tile([128, kk, tin, NB], f32, tag="big")
                for k in range(kk):
                    for c in range(HC):
                        nc.tensor.matmul(
                            pt[:, k, :, :],
                            wtile[:, c, m * 128:(m + 1) * 128, k],
                            src[:, c, 0:tin * NB],
                            start=(c == 0), stop=(c == HC - 1))
                ps.append(pt)
            out = acts.tile([128, HC, tout * NB], dec_dt, tag=out_tag)
            W = 8 + extra_w
            pay = small.tile([128, W], f32, tag="pay")
            stats_to_pay(ps, pay, tout * NB)
            if extra is not None:
                nc.vector.tensor_copy(pay[:, 8:W], extra)
            gpay = gather_pay(pay, W)
            scale, bias = bn_finalize(gpay, float(NCORES * tout * NB),
                                      f"g_{lname}", f"b_{lname}")
            for m in range(HC):
                oap = out[:, m, :].rearrange("p (l k b) -> p k l b",
                                             l=tin, k=kk)
                nc.scalar.activation(oap, ps[m][:, :, :], AF.Relu,
                                     bias=bias[:, m:m + 1],
                                     scale=scale[:, m:m + 1])
            return out, gpay

        wd1 = wload(wdec["w_d1"], 2, dec_dt)
        extra = small.tile([128, 5], f32, tag="extra4")
        nc.vector.tensor_copy(extra[:, 0:4], hist[:, :])
        nc.vector.tensor_copy(extra[:, 4:5], vqcol[:, :])
        d1, gpay4 = conv_dec(d0, wd1, 2, 1, "d1", "mid2", extra=extra[:, :], extra_w=5)
        hist_g = small.tile([128, HC], f32, tag="hist_g")
        vq_g = small.tile([128, 1], f32, tag="vq_g")
        nc.vector.tensor_copy(hist_g[:, :], gpay4[:, 8:12])
        nc.vector.tensor_copy(vq_g[:, :], gpay4[:, 12:13])

        wd2 = wload(wdec["w_d2"], 3, dec_dt)
        d2, _ = conv_dec(d1, wd2, 3, 2, "d2", "mid1")

        # ---- d3: too big for PSUM residence; stats read PSUM, raw -> SBUF ----
        scope("d3")
        wd3 = wload(wdec["w_d3"], 5, dec_dt)
        d3raw = acts.tile([128, HC, T * NB], dec_dt, tag="big1")  # reuse h0 slot
        sump3 = consts.tile([128, HC, 3, 3], f32, tag="st12")
        sqp3 = consts.tile([128, HC, 3, 3], f32, tag="mv43")
        for m in range(HC):
            d3r_m = d3raw[:, m, :].rearrange("p (l k b) -> p k l b", l=6, k=5)
            for lp in range(3):          # l-pairs (2 l's per matmul, N=256)
                lsl = slice(2 * lp, 2 * lp + 2)
                rsrc = d2[:, :, 2 * lp * NB:(2 * lp + 2) * NB]
                pt_a = psp.tile([128, 2, 2, NB], f32, tag="big")
                pt_b = psp.tile([128, 2, 2, NB], f32, tag="big")
                pt_c = psp.tile([128, 1, 2, NB], f32, tag="big")
                pts = [pt_a, pt_b, pt_c]
                for k in range(5):
                    dst = pts[k // 2][:, k % 2, :, :]
                    for c in range(HC):
                        nc.tensor.matmul(
                            dst,
                            wd3[:, c, m * 128:(m + 1) * 128, k],
                            rsrc[:, c, :],
                            start=(c == 0), stop=(c == HC - 1))
                for j, pt in enumerate(pts):
                    nk = pt.shape[1]
                    sqsc3 = acts.tile([128, 2 * 2 * NB], f32, tag="sqsc")
                    nc.scalar.activation(sqsc3[:, 0:nk * 2 * NB], pt[:, :, :, :],
                                         AF.Square,
                                         accum_out=sqp3[:, m, lp, j:j + 1])
                    nc.vector.tensor_scalar(
                        out=d3r_m[:, 2 * j:2 * j + nk, lsl, :],
                        in0=pt[:, :, :, :],
                        scalar1=0.0, scalar2=0.0,
                        op0=ALU.add, op1=ALU.add,
                        accum_out=sump3[:, m, lp, j:j + 1])
        pay = small.tile([128, 8], f32, tag="pay")
        AXY = mybir.AxisListType.XY
        for m in range(HC):
            nc.vector.tensor_reduce(pay[:, m:m + 1], sump3[:, m, :, :],
                                    axis=AXY, op=ALU.add)
            nc.vector.tensor_reduce(pay[:, 4 + m:5 + m], sqp3[:, m, :, :],
                                    axis=AXY, op=ALU.add)
        gpay = gather_pay(pay, 8)
        scale, bias = bn_finalize(gpay, float(NCORES * T * NB), "g_d3", "b_d3")
        d3 = d3raw

        # ---- recon = W_out.T @ d3 + b_out (apply sliced n-major so the
        #      recon matmuls chase the BN applies) ----
        scope("recon")
        NR = 480
        for n in range(T * NB // NR):
            sl = slice(n * NR, (n + 1) * NR)
            for m in range(HC):
                nc.scalar.activation(d3[:, m, sl], d3raw[:, m, sl], AF.Relu,
                                     bias=bias[:, m:m + 1],
                                     scale=scale[:, m:m + 1])
            for mlo, msz, bt in [(0, 128, bout0), (128, 7, bout1)]:
                rp = psp.tile([128, NR], f32, tag="big")
                for c in range(HC):
                    nc.tensor.matmul(rp[0:msz, :], wout[:, c, mlo:mlo + msz],
                                     d3[:, c, sl],
                                     start=(c == 0), stop=(c == HC - 1))
                rs = rec.tile([128, NR], f32, tag="recsb")
                nc.vector.tensor_scalar_add(out=rs[0:msz, :], in0=rp[0:msz, :],
                                            scalar1=bt[:, 0:1])
                nc.sync.dma_start(recon_d[mlo:mlo + msz, sl], rs[0:msz, :])

        # ---- scalars: vq_loss & perplexity ----
        vps = psp.tile([1, 1], f32, tag="big")
        nc.tensor.matmul(vps[:, :], vq_g[:, 0:1], onesP[:, :], start=True, stop=True)
        scl = small.tile([1, 2], f32, tag="scl")
        nc.scalar.mul(scl[0:1, 0:1], vps[:, :], 1.25 / float(B * H))
        hist_gc = rec.tile([128, HC], f32, tag="recsb")
        nc.vector.tensor_copy(hist_gc[:, :], hist_g[:, :])
        p_t = small.tile([128, HC], f32, tag="p_t")
        lnp = small.tile([128, HC], f32, tag="lnp")
        nc.scalar.mul(p_t[:, :], hist_gc[:, :], 1.0 / float(B))
        eps10 = small.tile([128, 1], f32, tag="eps10")
        nc.vector.memset(eps10[:, :], 1e-10)
        nc.scalar.activation(lnp[:, :], hist_gc[:, :], AF.Ln,
                             bias=eps10[:, 0:1], scale=1.0 / float(B))
        nc.vector.tensor_mul(p_t[:, :], p_t[:, :], lnp[:, :])
        ent = small.tile([128, 1], f32, tag="ent")
        nc.vector.reduce_sum(ent[:, 0:1], p_t[:, :], axis=AX)
        entp = psp.tile([1, 1], f32, tag="big")
        nc.tensor.matmul(entp[:, :], ent[:, 0:1], onesP[:, :], start=True, stop=True)
        nc.scalar.activation(scl[0:1, 1:2], entp[:, :], AF.Exp, scale=-1.0)
        nc.sync.dma_start(vq_d[:, :], scl[0:1, :])
        scope("end")
        nc.leave_named_scope(_sc[0][0], _sc[0][1], False)

    nc.compile()
    return nc


def _prep_inputs(inputs):
    f = np.float32
    dec = np.float16 if DEC_FP16 else np.float32
    x = np.asarray(inputs["input_seqs"], f)
    chvec = np.stack([np.asarray(inputs[n], f).reshape(HC, 128)
                      for n in CHV]).astype(f)
    common = {
        "w_in": np.ascontiguousarray(np.asarray(inputs["W_in"], f)),
        "chvec": np.ascontiguousarray(chvec),
        "b_out": np.ascontiguousarray(np.asarray(inputs["b_out"], f).reshape(D, 1)),
        "codebook": np.ascontiguousarray(np.asarray(inputs["codebook"], f)),
        "codebook_t": np.ascontiguousarray(np.asarray(inputs["codebook"], f).T),
        "w_q": np.ascontiguousarray(np.asarray(inputs["W_q"], f).astype(dec)),
        "w_out": np.ascontiguousarray(np.asarray(inputs["W_out"], f).astype(dec)),
        "w_e1": np.ascontiguousarray(np.asarray(inputs["w_e1"], f).transpose(1, 0, 2)),
        "w_e2": np.ascontiguousarray(np.asarray(inputs["w_e2"], f).transpose(1, 0, 2)),
        "w_e3": np.ascontiguousarray(np.asarray(inputs["w_e3"], f).transpose(1, 0, 2)),
        "w_d1": np.ascontiguousarray(np.asarray(inputs["w_d1"], f).astype(dec)),
        "w_d2": np.ascontiguousarray(np.asarray(inputs["w_d2"], f).astype(dec)),
        "w_d3": np.ascontiguousarray(np.asarray(inputs["w_d3"], f).astype(dec)),
    }
    in_maps = []
    for c in range(NCORES):
        xc = np.ascontiguousarray(
            x[:, c * BL:(c + 1) * BL, :].transpose(2, 0, 1).reshape(D, T * BL))
        m = {"xin": xc}
        m.update(common)
        in_maps.append(m)
    return in_maps


def kernel(**inputs):
    from concourse.bass_utils import run_bass_kernel_spmd
    if "nc" not in _CACHE:
        _CACHE["nc"] = _build()
    nc = _CACHE["nc"]
    in_maps = _prep_inputs(inputs)
    for attempt in range(3):
        res = run_bass_kernel_spmd(nc, in_maps, core_ids=list(range(NCORES)))
        recon = np.empty((B, T, D), np.float32)
        for c in range(NCORES):
            rc = res.results[c]["recon"].reshape(D, T, BL)
            recon[c * BL:(c + 1) * BL] = rc.transpose(2, 1, 0)
        vqs = np.stack([res.results[c]["vq"] for c in range(NCORES)])
        # integrity: scalars are AllReduced, must agree on every core;
        # NaNs indicate a transient device fault -> rerun
        ok = (np.isfinite(recon).all() and np.isfinite(vqs).all()
              and all(np.array_equal(vqs[c], vqs[0]) for c in range(NCORES)))
        if ok:
            break
    vq = vqs[0]
    return recon, np.float32(vq[0, 0]), np.float32(vq[0, 1])
